# revision 14
# baseline (speedup 1.0000x reference)
"""Trainium2 Bass kernel for Llama4TextExperts (MoE expert MLP chain).

Problem: E=8 experts, T=2048 tokens/expert, H=2048 hidden, D=4096 intermediate.
  hs (E*T, H) -> per expert e: g = hs_e @ Wg_e; u = hs_e @ Wu_e;
  f = u * silu(g); y_e = f @ Wd_e  -> out (E*T, H), all fp32.

Sharding: expert-parallel, 1 expert per NeuronCore (8 cores).

Per-core kernel design (v3):
  - All matmul operands bf16 (measured rel err ~3.7e-3 vs fp64; gate 2e-2).
  - Host pre-transposes hs_e -> xT [H, T]; all of xT resident in SBUF
    (8.4MB), chunk-DMAed so the first t-tile's slices land first.
  - Loop over T in tiles of TT=512 tokens (one PSUM bank per matmul,
    contiguous accumulation groups -- bank alternation between
    consecutive matmuls measurably breaks LDWEIGHTS pull-ahead):
      stage 1: per d-tile (128 wide): psum_g/psum_u [128, 512] accumulate
        16 matmuls over h-chunks (lhsT = W[h,d] 128x128 stationary,
        rhs = xT[h, t-tile] 128x512 moving). silu on ScalarE,
        f = silu(g)*u on VectorE -> f[dt] SBUF [128(d) x 512(t)] bf16.
      stage 2: computed as y^T: per 128-wide h-block, psum_y [128(h) x
        512(t)] accumulates 32 matmuls over d (lhsT = wd[d,h] 128x128
        stationary, rhs = f[dt] 128x512 moving). ScalarE copy -> DMA to
        y [H, T] (y^T layout; host transposes back).
  - Per-core DMA ~160MB total; weight streams double-buffered under
    compute.
"""

import os
import sys

for _p in ("/opt/trn_rl_repo",):
    if _p not in sys.path and os.path.isdir(_p):
        sys.path.insert(0, _p)

import numpy as np
from ml_dtypes import bfloat16 as bf16

E = 8
T = 2048
H = 2048
D = 4096

_CACHE = {}


def _build_bass(H_=H, D_=D, T_=T, TT=512):
    """Build the single-core Bass module (same program for all 8 cores)."""
    import concourse.bass as bass
    import concourse.mybir as mybir
    from concourse.tile import TileContext

    f32 = mybir.dt.float32
    bf = mybir.dt.bfloat16
    P = 128
    N_H = H_ // P            # h-chunks (16)
    N_D = D_ // P            # d-tiles (32)
    N_TT = T_ // TT          # t-tiles (4)
    WGD = 256                # wg/wu d-width per load (2 d-tiles)

    nc = bass.Bass(trn_type="TRN2")

    xT = nc.declare_dram_parameter("xT", [H_, T_], bf, isOutput=False)
    wg = nc.declare_dram_parameter("wg", [H_, D_], bf, isOutput=False)
    wu = nc.declare_dram_parameter("wu", [H_, D_], bf, isOutput=False)
    wd = nc.declare_dram_parameter("wd", [D_, H_], bf, isOutput=False)
    y = nc.declare_dram_parameter("y", [H_, T_], f32, isOutput=True)  # y^T

    xT_r = xT[:].rearrange("(n p) t -> p n t", p=P)    # [128, N_H, T]
    wg_r = wg[:].rearrange("(n p) d -> p n d", p=P)    # [128, N_H, D]
    wu_r = wu[:].rearrange("(n p) d -> p n d", p=P)
    wd_r = wd[:].rearrange("(n p) h -> p n h", p=P)    # [128, N_D, H]
    y_r = y[:].rearrange("(n p) t -> p n t", p=P)      # [128, N_H, T]

    with TileContext(nc) as tc:
        with (
            tc.tile_pool(name="xpool", bufs=1) as xpool,
            tc.tile_pool(name="wpool", bufs=2) as wpool,
            tc.tile_pool(name="wdpool", bufs=2) as wdpool,
            tc.tile_pool(name="fpool", bufs=N_D) as fpool,
            tc.tile_pool(name="spool", bufs=2) as spool,
            tc.tile_pool(name="ypool", bufs=4) as ypool,
            tc.tile_pool(name="pgu", bufs=2, space="PSUM") as pgu,
            tc.tile_pool(name="py", bufs=4, space="PSUM") as py,
        ):
            # ---- load ALL of xT once; first t-tile's slices first (and
            # finer-grained) so stage 1 can start as soon as possible.
            x_all = xpool.tile([P, N_H, T_], bf, tag="x")
            for tc_ in range(N_TT):
                nchunk = 4 if tc_ == 0 else 2
                hw_ = N_H // nchunk
                for hh in range(nchunk):
                    nc.scalar.dma_start(
                        out=x_all[:, hh * hw_:(hh + 1) * hw_,
                                  tc_ * TT:(tc_ + 1) * TT],
                        in_=xT_r[:, hh * hw_:(hh + 1) * hw_,
                                 tc_ * TT:(tc_ + 1) * TT],
                    )

            for tt in range(N_TT):
                tsl = slice(tt * TT, (tt + 1) * TT)

                # ---- stage 1: gate/up + swiglu, d-tile at a time
                f_tiles = []
                for dt in range(N_D):
                    dw = dt % (WGD // P)   # position inside current weight load
                    if dw == 0:
                        dsl = slice(dt * P, dt * P + WGD)
                        wg_t = wpool.tile([P, N_H, WGD], bf, tag="wg")
                        wu_t = wpool.tile([P, N_H, WGD], bf, tag="wu")
                        nc.sync.dma_start(out=wg_t, in_=wg_r[:, :, dsl])
                        nc.sync.dma_start(out=wu_t, in_=wu_r[:, :, dsl])
                    psum_g = pgu.tile([P, TT], f32, tag="pg")
                    psum_u = pgu.tile([P, TT], f32, tag="pu")
                    for h in range(N_H):
                        nc.tensor.matmul(
                            psum_g,
                            lhsT=wg_t[:, h, dw * P:(dw + 1) * P],
                            rhs=x_all[:, h, tsl],
                            start=(h == 0), stop=(h == N_H - 1),
                        )
                    for h in range(N_H):
                        nc.tensor.matmul(
                            psum_u,
                            lhsT=wu_t[:, h, dw * P:(dw + 1) * P],
                            rhs=x_all[:, h, tsl],
                            start=(h == 0), stop=(h == N_H - 1),
                        )
                    s_t = spool.tile([P, TT], f32, tag="s")
                    nc.scalar.activation(
                        out=s_t, in_=psum_g,
                        func=mybir.ActivationFunctionType.Silu,
                    )
                    f_t = fpool.tile([P, TT], bf, tag="f")
                    nc.vector.tensor_mul(f_t, s_t, psum_u)
                    f_tiles.append(f_t)

                # ---- stage 2: y^T[hb] = sum_dt wd[dt, hb].T @ f[dt]
                for hb in range(N_H):
                    wd_t = wdpool.tile([P, N_D, P], bf, tag="wd")
                    nc.scalar.dma_start(
                        out=wd_t,
                        in_=wd_r[:, :, hb * P:(hb + 1) * P],
                    )
                    psum_y = py.tile([P, TT], f32, tag="py")
                    for dt in range(N_D):
                        nc.tensor.matmul(
                            psum_y,
                            lhsT=wd_t[:, dt, :],
                            rhs=f_tiles[dt][:, :],
                            start=(dt == 0), stop=(dt == N_D - 1),
                        )
                    for half in range(2):
                        hsl = slice(half * (TT // 2), (half + 1) * (TT // 2))
                        y_sb = ypool.tile([P, TT // 2], f32, tag="y")
                        nc.scalar.copy(out=y_sb, in_=psum_y[:, hsl])
                        nc.scalar.dma_start(
                            out=y_r[:, hb, tt * TT + half * (TT // 2):
                                    tt * TT + (half + 1) * (TT // 2)],
                            in_=y_sb,
                        )
    _split_matmul_waits(nc)
    return nc


def _split_matmul_waits(nc, hoist_depth=0):
    """walrus splits Matmult into LDW+MM and moves the Matmult's sync
    waits onto the generated LW struct, which has room for only one wait.
    Hoist every Matmult's waits onto a PE InstNoOp placed `hoist_depth`
    PE-instructions EARLIER in the stream: a wait-nop sitting directly in
    front of a matmul blocks the LDWEIGHTS pull-ahead for one MM slot
    (measured ~213ns x ~128 weight-tile boundaries); placed earlier, the
    (always already satisfied) wait clears while prior matmuls stream.
    Moving a wait earlier is strictly safe for ordering; deadlock-free as
    long as hoist_depth is smaller than the buffer-free windows (32-64
    matmuls for all pools here)."""
    import concourse.mybir as mybir

    for f in nc.m.functions:
        for bb in f.blocks:
            insts = list(bb.instructions)
            out = []
            pe_positions = []   # indices in `out` of PE-engine instructions
            n_nops = 0
            for ins in insts:
                si = ins.sync_info
                tname = type(ins).__name__
                if (
                    si is not None
                    and len(si.on_wait) > (1 if tname != "InstMatmult" else 0)
                ):
                    keep = [] if tname == "InstMatmult" else [si.on_wait[-1]]
                    hoist = si.on_wait if tname == "InstMatmult" else si.on_wait[:-1]
                    nops = []
                    for i, w in enumerate(hoist):
                        nops.append(mybir.InstNoOp(
                            name=f"{ins.name}-waitnop{i}",
                            engine=ins.engine,
                            ins=[],
                            outs=[],
                            sync_info=mybir.SyncInfo(
                                on_wait=[w], on_update=[]
                            ),
                        ))
                        n_nops += 1
                    if hoist_depth > 0 and ins.engine == mybir.EngineType.PE \
                            and len(pe_positions) >= hoist_depth:
                        # insert before the hoist_depth-th previous PE inst
                        pos = pe_positions[-hoist_depth]
                        out[pos:pos] = nops
                        pe_positions = [
                            p if p < pos else p + len(nops)
                            for p in pe_positions
                        ]
                    else:
                        out.extend(nops)
                    ins.sync_info = mybir.SyncInfo(
                        on_wait=keep, on_update=list(si.on_update)
                    )
                if ins.engine == mybir.EngineType.PE:
                    pe_positions.append(len(out))
                out.append(ins)
            if n_nops:
                bb.instructions = out


def make_in_maps(hidden_states, gate_proj, up_proj, down_proj):
    hs = np.ascontiguousarray(hidden_states, dtype=np.float32).reshape(E, T, H)
    in_maps = []
    for e in range(E):
        in_maps.append({
            "xT": np.ascontiguousarray(hs[e].T).astype(bf16),
            "wg": np.ascontiguousarray(gate_proj[e], dtype=np.float32).astype(bf16),
            "wu": np.ascontiguousarray(up_proj[e], dtype=np.float32).astype(bf16),
            "wd": np.ascontiguousarray(down_proj[e], dtype=np.float32).astype(bf16),
        })
    return in_maps


def kernel(hidden_states, gate_proj, up_proj, down_proj):
    from concourse.bass_utils import run_bass_kernel_spmd

    in_maps = make_in_maps(hidden_states, gate_proj, up_proj, down_proj)
    if "nc" not in _CACHE:
        _CACHE["nc"] = _build_bass()
    nc = _CACHE["nc"]

    res = run_bass_kernel_spmd(nc, in_maps, core_ids=list(range(E)))
    # y comes back as y^T [H, T] per expert
    out = np.concatenate(
        [np.ascontiguousarray(res.results[e]["y"].T) for e in range(E)], axis=0
    )
    return out.astype(np.float32)


if __name__ == "__main__":
    # smoke: build only
    nc = _build_bass()
    print("built ok, instructions:", len(nc.inst_map))


# revision 15
# speedup vs baseline: 1.0188x; 1.0188x over previous
"""Trainium2 Bass kernel for Llama4TextExperts (MoE expert MLP chain).

Problem: E=8 experts, T=2048 tokens/expert, H=2048 hidden, D=4096 intermediate.
  hs (E*T, H) -> per expert e: g = hs_e @ Wg_e; u = hs_e @ Wu_e;
  f = u * silu(g); y_e = f @ Wd_e  -> out (E*T, H), all fp32.

Sharding: expert-parallel, 1 expert per NeuronCore (8 cores).

Per-core kernel design (v3):
  - All matmul operands bf16 (measured rel err ~3.7e-3 vs fp64; gate 2e-2).
  - Host pre-transposes hs_e -> xT [H, T]; all of xT resident in SBUF
    (8.4MB), chunk-DMAed so the first t-tile's slices land first.
  - Loop over T in tiles of TT=512 tokens (one PSUM bank per matmul,
    contiguous accumulation groups -- bank alternation between
    consecutive matmuls measurably breaks LDWEIGHTS pull-ahead):
      stage 1: per d-tile (128 wide): psum_g/psum_u [128, 512] accumulate
        16 matmuls over h-chunks (lhsT = W[h,d] 128x128 stationary,
        rhs = xT[h, t-tile] 128x512 moving). silu on ScalarE,
        f = silu(g)*u on VectorE -> f[dt] SBUF [128(d) x 512(t)] bf16.
      stage 2: computed as y^T: per 128-wide h-block, psum_y [128(h) x
        512(t)] accumulates 32 matmuls over d (lhsT = wd[d,h] 128x128
        stationary, rhs = f[dt] 128x512 moving). ScalarE copy -> DMA to
        y [H, T] (y^T layout; host transposes back).
  - Per-core DMA ~160MB total; weight streams double-buffered under
    compute.
"""

import os
import sys

for _p in ("/opt/trn_rl_repo",):
    if _p not in sys.path and os.path.isdir(_p):
        sys.path.insert(0, _p)

import numpy as np
from ml_dtypes import bfloat16 as bf16

E = 8
T = 2048
H = 2048
D = 4096

_CACHE = {}


def _build_bass(H_=H, D_=D, T_=T, TT=512):
    """Build the single-core Bass module (same program for all 8 cores)."""
    import concourse.bass as bass
    import concourse.mybir as mybir
    from concourse.tile import TileContext

    f32 = mybir.dt.float32
    bf = mybir.dt.bfloat16
    P = 128
    N_H = H_ // P            # h-chunks (16)
    N_D = D_ // P            # d-tiles (32)
    N_TT = T_ // TT          # t-tiles (4)
    WGD = 256                # wg/wu d-width per load (2 d-tiles)

    nc = bass.Bass(trn_type="TRN2")

    xT = nc.declare_dram_parameter("xT", [H_, T_], bf, isOutput=False)
    wg = nc.declare_dram_parameter("wg", [H_, D_], bf, isOutput=False)
    wu = nc.declare_dram_parameter("wu", [H_, D_], bf, isOutput=False)
    wd = nc.declare_dram_parameter("wd", [D_, H_], bf, isOutput=False)
    y = nc.declare_dram_parameter("y", [H_, T_], f32, isOutput=True)  # y^T

    xT_r = xT[:].rearrange("(n p) t -> p n t", p=P)    # [128, N_H, T]
    wg_r = wg[:].rearrange("(n p) d -> p n d", p=P)    # [128, N_H, D]
    wu_r = wu[:].rearrange("(n p) d -> p n d", p=P)
    wd_r = wd[:].rearrange("(n p) h -> p n h", p=P)    # [128, N_D, H]
    y_r = y[:].rearrange("(n p) t -> p n t", p=P)      # [128, N_H, T]

    with TileContext(nc) as tc:
        with (
            tc.tile_pool(name="xpool", bufs=1) as xpool,
            tc.tile_pool(name="wpool", bufs=2) as wpool,
            tc.tile_pool(name="wdpool", bufs=2) as wdpool,
            tc.tile_pool(name="fpool", bufs=N_D) as fpool,
            tc.tile_pool(name="spool", bufs=2) as spool,
            tc.tile_pool(name="ypool", bufs=4) as ypool,
            tc.tile_pool(name="pgu", bufs=2, space="PSUM") as pgu,
            tc.tile_pool(name="py", bufs=4, space="PSUM") as py,
        ):
            # ---- load ALL of xT once; first t-tile's slices first (and
            # finer-grained) so stage 1 can start as soon as possible.
            x_all = xpool.tile([P, N_H, T_], bf, tag="x")
            for tc_ in range(N_TT):
                for hh in range(2):
                    nc.scalar.dma_start(
                        out=x_all[:, hh * 8:(hh + 1) * 8,
                                  tc_ * TT:(tc_ + 1) * TT],
                        in_=xT_r[:, hh * 8:(hh + 1) * 8,
                                 tc_ * TT:(tc_ + 1) * TT],
                    )

            for tt in range(N_TT):
                tsl = slice(tt * TT, (tt + 1) * TT)

                # ---- stage 1: gate/up + swiglu, d-tile at a time
                f_tiles = []
                for dt in range(N_D):
                    dw = dt % (WGD // P)   # position inside current weight load
                    if dw == 0:
                        dsl = slice(dt * P, dt * P + WGD)
                        wg_t = wpool.tile([P, N_H, WGD], bf, tag="wg")
                        wu_t = wpool.tile([P, N_H, WGD], bf, tag="wu")
                        nc.sync.dma_start(out=wg_t, in_=wg_r[:, :, dsl])
                        nc.sync.dma_start(out=wu_t, in_=wu_r[:, :, dsl])
                    psum_g = pgu.tile([P, TT], f32, tag="pg")
                    psum_u = pgu.tile([P, TT], f32, tag="pu")
                    for h in range(N_H):
                        nc.tensor.matmul(
                            psum_g,
                            lhsT=wg_t[:, h, dw * P:(dw + 1) * P],
                            rhs=x_all[:, h, tsl],
                            start=(h == 0), stop=(h == N_H - 1),
                        )
                    for h in range(N_H):
                        nc.tensor.matmul(
                            psum_u,
                            lhsT=wu_t[:, h, dw * P:(dw + 1) * P],
                            rhs=x_all[:, h, tsl],
                            start=(h == 0), stop=(h == N_H - 1),
                        )
                    s_t = spool.tile([P, TT], f32, tag="s")
                    nc.scalar.activation(
                        out=s_t, in_=psum_g,
                        func=mybir.ActivationFunctionType.Silu,
                    )
                    f_t = fpool.tile([P, TT], bf, tag="f")
                    nc.vector.tensor_mul(f_t, s_t, psum_u)
                    f_tiles.append(f_t)

                # ---- stage 2: y^T[hb] = sum_dt wd[dt, hb].T @ f[dt]
                for hb in range(N_H):
                    wd_t = wdpool.tile([P, N_D, P], bf, tag="wd")
                    nc.scalar.dma_start(
                        out=wd_t,
                        in_=wd_r[:, :, hb * P:(hb + 1) * P],
                    )
                    psum_y = py.tile([P, TT], f32, tag="py")
                    for dt in range(N_D):
                        nc.tensor.matmul(
                            psum_y,
                            lhsT=wd_t[:, dt, :],
                            rhs=f_tiles[dt][:, :],
                            start=(dt == 0), stop=(dt == N_D - 1),
                        )
                    y_sb = ypool.tile([P, TT], f32, tag="y")
                    nc.scalar.copy(out=y_sb, in_=psum_y)
                    nc.scalar.dma_start(out=y_r[:, hb, tsl], in_=y_sb)
    _split_matmul_waits(nc)
    return nc


def _split_matmul_waits(nc, hoist_depth=0):
    """walrus splits Matmult into LDW+MM and moves the Matmult's sync
    waits onto the generated LW struct, which has room for only one wait.
    Hoist every Matmult's waits onto a PE InstNoOp placed `hoist_depth`
    PE-instructions EARLIER in the stream: a wait-nop sitting directly in
    front of a matmul blocks the LDWEIGHTS pull-ahead for one MM slot
    (measured ~213ns x ~128 weight-tile boundaries); placed earlier, the
    (always already satisfied) wait clears while prior matmuls stream.
    Moving a wait earlier is strictly safe for ordering; deadlock-free as
    long as hoist_depth is smaller than the buffer-free windows (32-64
    matmuls for all pools here)."""
    import concourse.mybir as mybir

    for f in nc.m.functions:
        for bb in f.blocks:
            insts = list(bb.instructions)
            out = []
            pe_positions = []   # indices in `out` of PE-engine instructions
            n_nops = 0
            for ins in insts:
                si = ins.sync_info
                tname = type(ins).__name__
                if (
                    si is not None
                    and len(si.on_wait) > (1 if tname != "InstMatmult" else 0)
                ):
                    keep = [] if tname == "InstMatmult" else [si.on_wait[-1]]
                    hoist = si.on_wait if tname == "InstMatmult" else si.on_wait[:-1]
                    nops = []
                    for i, w in enumerate(hoist):
                        nops.append(mybir.InstNoOp(
                            name=f"{ins.name}-waitnop{i}",
                            engine=ins.engine,
                            ins=[],
                            outs=[],
                            sync_info=mybir.SyncInfo(
                                on_wait=[w], on_update=[]
                            ),
                        ))
                        n_nops += 1
                    if hoist_depth > 0 and ins.engine == mybir.EngineType.PE \
                            and len(pe_positions) >= hoist_depth:
                        # insert before the hoist_depth-th previous PE inst
                        pos = pe_positions[-hoist_depth]
                        out[pos:pos] = nops
                        pe_positions = [
                            p if p < pos else p + len(nops)
                            for p in pe_positions
                        ]
                    else:
                        out.extend(nops)
                    ins.sync_info = mybir.SyncInfo(
                        on_wait=keep, on_update=list(si.on_update)
                    )
                if ins.engine == mybir.EngineType.PE:
                    pe_positions.append(len(out))
                out.append(ins)
            if n_nops:
                bb.instructions = out


def make_in_maps(hidden_states, gate_proj, up_proj, down_proj):
    hs = np.ascontiguousarray(hidden_states, dtype=np.float32).reshape(E, T, H)
    in_maps = []
    for e in range(E):
        in_maps.append({
            "xT": np.ascontiguousarray(hs[e].T).astype(bf16),
            "wg": np.ascontiguousarray(gate_proj[e], dtype=np.float32).astype(bf16),
            "wu": np.ascontiguousarray(up_proj[e], dtype=np.float32).astype(bf16),
            "wd": np.ascontiguousarray(down_proj[e], dtype=np.float32).astype(bf16),
        })
    return in_maps


def kernel(hidden_states, gate_proj, up_proj, down_proj):
    from concourse.bass_utils import run_bass_kernel_spmd

    in_maps = make_in_maps(hidden_states, gate_proj, up_proj, down_proj)
    if "nc" not in _CACHE:
        _CACHE["nc"] = _build_bass()
    nc = _CACHE["nc"]

    res = run_bass_kernel_spmd(nc, in_maps, core_ids=list(range(E)))
    # y comes back as y^T [H, T] per expert
    out = np.concatenate(
        [np.ascontiguousarray(res.results[e]["y"].T) for e in range(E)], axis=0
    )
    return out.astype(np.float32)


if __name__ == "__main__":
    # smoke: build only
    nc = _build_bass()
    print("built ok, instructions:", len(nc.inst_map))


# revision 16
# speedup vs baseline: 1.0365x; 1.0173x over previous
"""Trainium2 Bass kernel for Llama4TextExperts (MoE expert MLP chain).

Problem: E=8 experts, T=2048 tokens/expert, H=2048 hidden, D=4096 intermediate.
  hs (E*T, H) -> per expert e: g = hs_e @ Wg_e; u = hs_e @ Wu_e;
  f = u * silu(g); y_e = f @ Wd_e  -> out (E*T, H), all fp32.

Sharding: expert-parallel, 1 expert per NeuronCore (8 cores).

Per-core kernel design (v10):
  - All matmul operands bf16 (measured rel err ~3.7e-3 vs fp64; gate 2e-2).
  - HOST pre-relayouts every streamed tensor so each DMA reads a single
    fully-contiguous 8-16KB run per partition (the natural [.,.,slice]
    patterns read 256-512B segments, which kept the DMA queues ~25%
    under rate and stalled the weight streams):
      x:  [128, N_TT, N_H, TT]  (one tile per t-tile; first matmul only
           waits on t-tile 0's x)
      wg/wu: [128, N_DG, N_H, WGD]
      wd: [128, N_H, N_D, 128]
  - Loop over T in tiles of TT=512 tokens (one PSUM bank per matmul;
    contiguous accumulation groups -- bank alternation between
    consecutive matmuls measurably breaks LDWEIGHTS pull-ahead):
      stage 1: per d-tile (128 wide): psum_g/psum_u [128, 512] accumulate
        16 matmuls over h-chunks (lhsT = W[h,d] 128x128 stationary,
        rhs = x[h, t-tile] 128x512 moving). silu on ScalarE,
        f = silu(g)*u on VectorE -> f[dt] SBUF [128(d) x 512(t)] bf16.
      stage 2: computed as y^T: per 128-wide h-block, psum_y [128(h) x
        512(t)] accumulates 32 matmuls over d (lhsT = wd[d,h] 128x128
        stationary, rhs = f[dt] 128x512 moving). ScalarE copy -> DMA to
        y [H, T] (y^T layout; host transposes back).
  - DMA queue split: wg/wu on the sync (SP) HW queue; x/wd/y on the
    scalar (ACT) HW queue. (gpsimd's queue is SWDGE -- measured 3x
    slower triggers, do not use.)
"""

import os
import sys

for _p in ("/opt/trn_rl_repo",):
    if _p not in sys.path and os.path.isdir(_p):
        sys.path.insert(0, _p)

import numpy as np
from ml_dtypes import bfloat16 as bf16

E = 8
T = 2048
H = 2048
D = 4096
P = 128
TT = 512
WGD = 256
N_H = H // P        # 16
N_D = D // P        # 32
N_TT = T // TT      # 4
N_DG = D // WGD     # 16

_CACHE = {}


def _build_bass():
    """Build the single-core Bass module (same program for all 8 cores)."""
    import concourse.bass as bass
    import concourse.mybir as mybir
    from concourse.tile import TileContext

    f32 = mybir.dt.float32
    bf = mybir.dt.bfloat16

    nc = bass.Bass(trn_type="TRN2")

    xT = nc.declare_dram_parameter("xT", [P, N_TT * N_H * TT], bf, isOutput=False)
    wg = nc.declare_dram_parameter("wg", [P, N_DG * N_H * WGD], bf, isOutput=False)
    wu = nc.declare_dram_parameter("wu", [P, N_DG * N_H * WGD], bf, isOutput=False)
    wd = nc.declare_dram_parameter("wd", [P, N_H * N_D * P], bf, isOutput=False)
    y = nc.declare_dram_parameter("y", [H, T], f32, isOutput=True)  # y^T

    x_r = xT[:].rearrange("p (t n c) -> p t n c", t=N_TT, n=N_H)
    wg_r = wg[:].rearrange("p (g n c) -> p g n c", g=N_DG, n=N_H)
    wu_r = wu[:].rearrange("p (g n c) -> p g n c", g=N_DG, n=N_H)
    wd_r = wd[:].rearrange("p (m n h) -> p m n h", m=N_H, n=N_D)
    y_r = y[:].rearrange("(n p) t -> p n t", p=P)      # [128, N_H, T]

    with TileContext(nc) as tc:
        with (
            tc.tile_pool(name="xpool", bufs=1) as xpool,
            tc.tile_pool(name="wpool", bufs=2) as wpool,
            tc.tile_pool(name="wdpool", bufs=2) as wdpool,
            tc.tile_pool(name="fpool", bufs=N_D) as fpool,
            tc.tile_pool(name="spool", bufs=2) as spool,
            tc.tile_pool(name="ypool", bufs=4) as ypool,
            tc.tile_pool(name="pgu", bufs=2, space="PSUM") as pgu,
            tc.tile_pool(name="py", bufs=4, space="PSUM") as py,
        ):
            # ---- load x once, one tile per t-tile (t-tile 0 first so
            # stage 1's first matmuls only wait for 2MB, not all 8.4MB)
            x_tiles = []
            for tt in range(N_TT):
                x_t = xpool.tile([P, N_H, TT], bf, tag=f"x{tt}")
                nc.scalar.dma_start(out=x_t, in_=x_r[:, tt])
                x_tiles.append(x_t)

            for tt in range(N_TT):
                tsl = slice(tt * TT, (tt + 1) * TT)

                # ---- stage 1: gate/up + swiglu, d-tile at a time
                f_tiles = []
                for dt in range(N_D):
                    dw = dt % (WGD // P)   # position inside current weight load
                    if dw == 0:
                        dg = dt // (WGD // P)
                        wg_t = wpool.tile([P, N_H, WGD], bf, tag="wg")
                        wu_t = wpool.tile([P, N_H, WGD], bf, tag="wu")
                        nc.sync.dma_start(out=wg_t, in_=wg_r[:, dg])
                        nc.sync.dma_start(out=wu_t, in_=wu_r[:, dg])
                    psum_g = pgu.tile([P, TT], f32, tag="pg")
                    psum_u = pgu.tile([P, TT], f32, tag="pu")
                    for h in range(N_H):
                        nc.tensor.matmul(
                            psum_g,
                            lhsT=wg_t[:, h, dw * P:(dw + 1) * P],
                            rhs=x_tiles[tt][:, h, :],
                            start=(h == 0), stop=(h == N_H - 1),
                        )
                    for h in range(N_H):
                        nc.tensor.matmul(
                            psum_u,
                            lhsT=wu_t[:, h, dw * P:(dw + 1) * P],
                            rhs=x_tiles[tt][:, h, :],
                            start=(h == 0), stop=(h == N_H - 1),
                        )
                    s_t = spool.tile([P, TT], f32, tag="s")
                    nc.scalar.activation(
                        out=s_t, in_=psum_g,
                        func=mybir.ActivationFunctionType.Silu,
                    )
                    f_t = fpool.tile([P, TT], bf, tag="f")
                    nc.vector.tensor_mul(f_t, s_t, psum_u)
                    f_tiles.append(f_t)

                # ---- stage 2: y^T[hb] = sum_dt wd[dt, hb].T @ f[dt]
                for hb in range(N_H):
                    wd_t = wdpool.tile([P, N_D, P], bf, tag="wd")
                    nc.scalar.dma_start(out=wd_t, in_=wd_r[:, hb])
                    psum_y = py.tile([P, TT], f32, tag="py")
                    for dt in range(N_D):
                        nc.tensor.matmul(
                            psum_y,
                            lhsT=wd_t[:, dt, :],
                            rhs=f_tiles[dt][:, :],
                            start=(dt == 0), stop=(dt == N_D - 1),
                        )
                    y_sb = ypool.tile([P, TT], f32, tag="y")
                    nc.scalar.copy(out=y_sb, in_=psum_y)
                    nc.scalar.dma_start(out=y_r[:, hb, tsl], in_=y_sb)
    _split_matmul_waits(nc)
    return nc


def _split_matmul_waits(nc):
    """walrus splits Matmult into LDW+MM and moves the Matmult's sync
    waits onto the generated LW struct, which has room for only one wait.
    Hoist every Matmult's waits onto a PE InstNoOp inserted just before it."""
    import concourse.mybir as mybir

    for f in nc.m.functions:
        for bb in f.blocks:
            insts = list(bb.instructions)
            out = []
            n_nops = 0
            for ins in insts:
                si = ins.sync_info
                tname = type(ins).__name__
                if (
                    si is not None
                    and len(si.on_wait) > (1 if tname != "InstMatmult" else 0)
                ):
                    keep = [] if tname == "InstMatmult" else [si.on_wait[-1]]
                    hoist = si.on_wait if tname == "InstMatmult" else si.on_wait[:-1]
                    for i, w in enumerate(hoist):
                        nop = mybir.InstNoOp(
                            name=f"{ins.name}-waitnop{i}",
                            engine=ins.engine,
                            ins=[],
                            outs=[],
                            sync_info=mybir.SyncInfo(
                                on_wait=[w], on_update=[]
                            ),
                        )
                        out.append(nop)
                        n_nops += 1
                    ins.sync_info = mybir.SyncInfo(
                        on_wait=keep, on_update=list(si.on_update)
                    )
                out.append(ins)
            if n_nops:
                bb.instructions = out


def make_in_maps(hidden_states, gate_proj, up_proj, down_proj):
    hs = np.ascontiguousarray(hidden_states, dtype=np.float32).reshape(E, T, H)
    in_maps = []
    for e in range(E):
        # x: [H, T] -> (n,p,tt,tc) -> (p,tt,n,tc), contiguous per partition
        xh = (hs[e].T.reshape(N_H, P, N_TT, TT)
              .transpose(1, 2, 0, 3).reshape(P, -1))
        # wg/wu: [H, D] -> (n,p,dg,dc) -> (p,dg,n,dc)
        wgh = (np.asarray(gate_proj[e], dtype=np.float32)
               .reshape(N_H, P, N_DG, WGD).transpose(1, 2, 0, 3).reshape(P, -1))
        wuh = (np.asarray(up_proj[e], dtype=np.float32)
               .reshape(N_H, P, N_DG, WGD).transpose(1, 2, 0, 3).reshape(P, -1))
        # wd: [D, H] -> (dt,p,hb,hc) -> (p,hb,dt,hc)
        wdh = (np.asarray(down_proj[e], dtype=np.float32)
               .reshape(N_D, P, N_H, P).transpose(1, 2, 0, 3).reshape(P, -1))
        in_maps.append({
            "xT": np.ascontiguousarray(xh).astype(bf16),
            "wg": np.ascontiguousarray(wgh).astype(bf16),
            "wu": np.ascontiguousarray(wuh).astype(bf16),
            "wd": np.ascontiguousarray(wdh).astype(bf16),
        })
    return in_maps


def kernel(hidden_states, gate_proj, up_proj, down_proj):
    from concourse.bass_utils import run_bass_kernel_spmd

    in_maps = make_in_maps(hidden_states, gate_proj, up_proj, down_proj)
    if "nc" not in _CACHE:
        _CACHE["nc"] = _build_bass()
    nc = _CACHE["nc"]

    res = run_bass_kernel_spmd(nc, in_maps, core_ids=list(range(E)))
    # y comes back as y^T [H, T] per expert
    out = np.concatenate(
        [np.ascontiguousarray(res.results[e]["y"].T) for e in range(E)], axis=0
    )
    return out.astype(np.float32)


if __name__ == "__main__":
    # smoke: build only
    nc = _build_bass()
    print("built ok, instructions:", len(nc.inst_map))


# revision 22
# speedup vs baseline: 1.0376x; 1.0011x over previous
"""Trainium2 Bass kernel for Llama4TextExperts (MoE expert MLP chain).

Problem: E=8 experts, T=2048 tokens/expert, H=2048 hidden, D=4096 intermediate.
  hs (E*T, H) -> per expert e: g = hs_e @ Wg_e; u = hs_e @ Wu_e;
  f = u * silu(g); y_e = f @ Wd_e  -> out (E*T, H), all fp32.

Sharding: expert-parallel, 1 expert per NeuronCore (8 cores).

Per-core kernel design (v10):
  - All matmul operands bf16 (measured rel err ~3.7e-3 vs fp64; gate 2e-2).
  - HOST pre-relayouts every streamed tensor so each DMA reads a single
    fully-contiguous 8-16KB run per partition (the natural [.,.,slice]
    patterns read 256-512B segments, which kept the DMA queues ~25%
    under rate and stalled the weight streams):
      x:  [128, N_TT, N_H, TT]  (one tile per t-tile; first matmul only
           waits on t-tile 0's x)
      wg/wu: [128, N_DG, N_H, WGD]
      wd: [128, N_H, N_D, 128]
  - Loop over T in tiles of TT=512 tokens (one PSUM bank per matmul;
    contiguous accumulation groups -- bank alternation between
    consecutive matmuls measurably breaks LDWEIGHTS pull-ahead):
      stage 1: per d-tile (128 wide): psum_g/psum_u [128, 512] accumulate
        16 matmuls over h-chunks (lhsT = W[h,d] 128x128 stationary,
        rhs = x[h, t-tile] 128x512 moving). silu on ScalarE,
        f = silu(g)*u on VectorE -> f[dt] SBUF [128(d) x 512(t)] bf16.
      stage 2: computed as y^T: per 128-wide h-block, psum_y [128(h) x
        512(t)] accumulates 32 matmuls over d (lhsT = wd[d,h] 128x128
        stationary, rhs = f[dt] 128x512 moving). ScalarE copy -> DMA to
        y [H, T] (y^T layout; host transposes back).
  - DMA queue split: wg/wu on the sync (SP) HW queue; x/wd/y on the
    scalar (ACT) HW queue. (gpsimd's queue is SWDGE -- measured 3x
    slower triggers, do not use.)
"""

import os
import sys

for _p in ("/opt/trn_rl_repo",):
    if _p not in sys.path and os.path.isdir(_p):
        sys.path.insert(0, _p)

import numpy as np
from ml_dtypes import bfloat16 as bf16

E = 8
T = 2048
H = 2048
D = 4096
P = 128
TT = 512
WGD = 256
N_H = H // P        # 16
N_D = D // P        # 32
N_TT = T // TT      # 4
N_DG = D // WGD     # 16

_CACHE = {}


def _build_bass():
    """Build the single-core Bass module (same program for all 8 cores)."""
    import concourse.bass as bass
    import concourse.mybir as mybir
    from concourse.tile import TileContext

    f32 = mybir.dt.float32
    bf = mybir.dt.bfloat16

    nc = bass.Bass(trn_type="TRN2")

    xT = nc.declare_dram_parameter("xT", [P, N_TT * N_H * TT], bf, isOutput=False)
    wg = nc.declare_dram_parameter("wg", [P, N_DG * N_H * WGD], bf, isOutput=False)
    wu = nc.declare_dram_parameter("wu", [P, N_DG * N_H * WGD], bf, isOutput=False)
    wd = nc.declare_dram_parameter("wd", [P, N_H * N_D * P], bf, isOutput=False)
    y = nc.declare_dram_parameter("y", [H, T], f32, isOutput=True)  # y^T

    x_r = xT[:].rearrange("p (t n c) -> p t n c", t=N_TT, n=N_H)
    wg_r = wg[:].rearrange("p (g n c) -> p g n c", g=N_DG, n=N_H)
    wu_r = wu[:].rearrange("p (g n c) -> p g n c", g=N_DG, n=N_H)
    wd_r = wd[:].rearrange("p (m n h) -> p m n h", m=N_H, n=N_D)
    y_r = y[:].rearrange("(n p) t -> p n t", p=P)      # [128, N_H, T]

    with TileContext(nc) as tc:
        with (
            tc.tile_pool(name="xpool", bufs=1) as xpool,
            tc.tile_pool(name="w0pool", bufs=1) as w0pool,
            tc.tile_pool(name="wpool", bufs=3) as wpool,
            tc.tile_pool(name="wdpool", bufs=3) as wdpool,
            tc.tile_pool(name="fpool", bufs=N_D) as fpool,
            tc.tile_pool(name="spool", bufs=2) as spool,
            tc.tile_pool(name="ypool", bufs=4) as ypool,
            tc.tile_pool(name="pgu", bufs=2, space="PSUM") as pgu,
            tc.tile_pool(name="py", bufs=4, space="PSUM") as py,
        ):
            # ---- load x once, one tile per t-tile (t-tile 0 first so
            # stage 1's first matmuls only wait for 2MB, not all 8.4MB).
            # t-tile 0 is further split in h-halves: the first matmul
            # then waits on only 1MB of x + 0.5MB of wg.
            x0a = xpool.tile([P, N_H // 2, TT], bf, tag="x0a")
            x0b = xpool.tile([P, N_H // 2, TT], bf, tag="x0b")
            nc.scalar.dma_start(out=x0a, in_=x_r[:, 0, 0:N_H // 2])
            nc.scalar.dma_start(out=x0b, in_=x_r[:, 0, N_H // 2:N_H])
            x_tiles = [None]
            for tt in range(1, N_TT):
                x_t = xpool.tile([P, N_H, TT], bf, tag=f"x{tt}")
                nc.scalar.dma_start(out=x_t, in_=x_r[:, tt])
                x_tiles.append(x_t)

            def x_rhs(tt, h):
                if tt == 0:
                    return (x0a if h < N_H // 2 else x0b)[:, h % (N_H // 2), :]
                return x_tiles[tt][:, h, :]

            # first wg/wu group (t-tile 0, dg 0) in h-halves, own pool
            # (bufs=1), so the first matmuls wait on 0.5MB weight loads
            HH = N_H // 2
            wg0a = w0pool.tile([P, HH, WGD], bf, tag="wg0a")
            wg0b = w0pool.tile([P, HH, WGD], bf, tag="wg0b")
            wu0a = w0pool.tile([P, HH, WGD], bf, tag="wu0a")
            wu0b = w0pool.tile([P, HH, WGD], bf, tag="wu0b")
            nc.sync.dma_start(out=wg0a, in_=wg_r[:, 0, 0:HH])
            nc.sync.dma_start(out=wg0b, in_=wg_r[:, 0, HH:N_H])
            nc.sync.dma_start(out=wu0a, in_=wu_r[:, 0, 0:HH])
            nc.sync.dma_start(out=wu0b, in_=wu_r[:, 0, HH:N_H])

            for tt in range(N_TT):
                tsl = slice(tt * TT, (tt + 1) * TT)

                # ---- stage 1: gate/up + swiglu, d-tile at a time
                f_tiles = []
                for dt in range(N_D):
                    dw = dt % (WGD // P)   # position inside current weight load
                    dg = dt // (WGD // P)
                    first_group = (tt == 0 and dg == 0)
                    if dw == 0 and not first_group:
                        wg_t = wpool.tile([P, N_H, WGD], bf, tag="wg")
                        wu_t = wpool.tile([P, N_H, WGD], bf, tag="wu")
                        nc.sync.dma_start(out=wg_t, in_=wg_r[:, dg])
                        nc.sync.dma_start(out=wu_t, in_=wu_r[:, dg])

                    def wlhs(w_t, wa, wb, h):
                        if first_group:
                            return (wa if h < HH else wb)[
                                :, h % HH, dw * P:(dw + 1) * P]
                        return w_t[:, h, dw * P:(dw + 1) * P]

                    psum_g = pgu.tile([P, TT], f32, tag="pg")
                    psum_u = pgu.tile([P, TT], f32, tag="pu")
                    for h in range(N_H):
                        nc.tensor.matmul(
                            psum_g,
                            lhsT=wlhs(None if first_group else wg_t,
                                      wg0a, wg0b, h),
                            rhs=x_rhs(tt, h),
                            start=(h == 0), stop=(h == N_H - 1),
                        )
                    for h in range(N_H):
                        nc.tensor.matmul(
                            psum_u,
                            lhsT=wlhs(None if first_group else wu_t,
                                      wu0a, wu0b, h),
                            rhs=x_rhs(tt, h),
                            start=(h == 0), stop=(h == N_H - 1),
                        )
                    s_t = spool.tile([P, TT], f32, tag="s")
                    nc.scalar.activation(
                        out=s_t, in_=psum_g,
                        func=mybir.ActivationFunctionType.Silu,
                    )
                    f_t = fpool.tile([P, TT], bf, tag="f")
                    nc.vector.tensor_mul(f_t, s_t, psum_u)
                    f_tiles.append(f_t)

                # ---- stage 2: y^T[hb] = sum_dt wd[dt, hb].T @ f[dt]
                for hb in range(N_H):
                    wd_t = wdpool.tile([P, N_D, P], bf, tag="wd")
                    nc.scalar.dma_start(out=wd_t, in_=wd_r[:, hb])
                    psum_y = py.tile([P, TT], f32, tag="py")
                    for dt in range(N_D):
                        nc.tensor.matmul(
                            psum_y,
                            lhsT=wd_t[:, dt, :],
                            rhs=f_tiles[dt][:, :],
                            start=(dt == 0), stop=(dt == N_D - 1),
                        )
                    # last eviction of the kernel: split in 4 so the
                    # copy->DMA drain pipelines instead of serializing
                    nsplit = 4 if (tt == N_TT - 1 and hb == N_H - 1) else 1
                    w_ = TT // nsplit
                    for sp in range(nsplit):
                        y_sb = ypool.tile([P, w_], f32,
                                          tag="y" if nsplit == 1 else "ylast")
                        nc.scalar.copy(
                            out=y_sb, in_=psum_y[:, sp * w_:(sp + 1) * w_])
                        nc.scalar.dma_start(
                            out=y_r[:, hb, tt * TT + sp * w_:
                                    tt * TT + (sp + 1) * w_],
                            in_=y_sb,
                        )
    _split_matmul_waits(nc)
    return nc


def _split_matmul_waits(nc):
    """walrus splits Matmult into LDW+MM and moves the Matmult's sync
    waits onto the generated LW struct, which has room for only one wait.
    Hoist every Matmult's waits onto a PE InstNoOp inserted just before it."""
    import concourse.mybir as mybir

    for f in nc.m.functions:
        for bb in f.blocks:
            insts = list(bb.instructions)
            out = []
            n_nops = 0
            for ins in insts:
                si = ins.sync_info
                tname = type(ins).__name__
                if (
                    si is not None
                    and len(si.on_wait) > (1 if tname != "InstMatmult" else 0)
                ):
                    keep = [] if tname == "InstMatmult" else [si.on_wait[-1]]
                    hoist = si.on_wait if tname == "InstMatmult" else si.on_wait[:-1]
                    for i, w in enumerate(hoist):
                        nop = mybir.InstNoOp(
                            name=f"{ins.name}-waitnop{i}",
                            engine=ins.engine,
                            ins=[],
                            outs=[],
                            sync_info=mybir.SyncInfo(
                                on_wait=[w], on_update=[]
                            ),
                        )
                        out.append(nop)
                        n_nops += 1
                    ins.sync_info = mybir.SyncInfo(
                        on_wait=keep, on_update=list(si.on_update)
                    )
                out.append(ins)
            if n_nops:
                bb.instructions = out


def make_in_maps(hidden_states, gate_proj, up_proj, down_proj):
    hs = np.ascontiguousarray(hidden_states, dtype=np.float32).reshape(E, T, H)
    in_maps = []
    for e in range(E):
        # x: [H, T] -> (n,p,tt,tc) -> (p,tt,n,tc), contiguous per partition
        xh = (hs[e].T.reshape(N_H, P, N_TT, TT)
              .transpose(1, 2, 0, 3).reshape(P, -1))
        # wg/wu: [H, D] -> (n,p,dg,dc) -> (p,dg,n,dc)
        wgh = (np.asarray(gate_proj[e], dtype=np.float32)
               .reshape(N_H, P, N_DG, WGD).transpose(1, 2, 0, 3).reshape(P, -1))
        wuh = (np.asarray(up_proj[e], dtype=np.float32)
               .reshape(N_H, P, N_DG, WGD).transpose(1, 2, 0, 3).reshape(P, -1))
        # wd: [D, H] -> (dt,p,hb,hc) -> (p,hb,dt,hc)
        wdh = (np.asarray(down_proj[e], dtype=np.float32)
               .reshape(N_D, P, N_H, P).transpose(1, 2, 0, 3).reshape(P, -1))
        in_maps.append({
            "xT": np.ascontiguousarray(xh).astype(bf16),
            "wg": np.ascontiguousarray(wgh).astype(bf16),
            "wu": np.ascontiguousarray(wuh).astype(bf16),
            "wd": np.ascontiguousarray(wdh).astype(bf16),
        })
    return in_maps


def kernel(hidden_states, gate_proj, up_proj, down_proj):
    from concourse.bass_utils import run_bass_kernel_spmd

    in_maps = make_in_maps(hidden_states, gate_proj, up_proj, down_proj)
    if "nc" not in _CACHE:
        _CACHE["nc"] = _build_bass()
    nc = _CACHE["nc"]

    res = run_bass_kernel_spmd(nc, in_maps, core_ids=list(range(E)))
    # y comes back as y^T [H, T] per expert
    out = np.concatenate(
        [np.ascontiguousarray(res.results[e]["y"].T) for e in range(E)], axis=0
    )
    return out.astype(np.float32)


if __name__ == "__main__":
    # smoke: build only
    nc = _build_bass()
    print("built ok, instructions:", len(nc.inst_map))


# revision 25
# speedup vs baseline: 1.0439x; 1.0060x over previous
"""Trainium2 Bass kernel for Llama4TextExperts (MoE expert MLP chain).

Problem: E=8 experts, T=2048 tokens/expert, H=2048 hidden, D=4096 intermediate.
  hs (E*T, H) -> per expert e: g = hs_e @ Wg_e; u = hs_e @ Wu_e;
  f = u * silu(g); y_e = f @ Wd_e  -> out (E*T, H), all fp32.

Sharding: expert-parallel, 1 expert per NeuronCore (8 cores).

Per-core kernel design (v10):
  - All matmul operands bf16 (measured rel err ~3.7e-3 vs fp64; gate 2e-2).
  - HOST pre-relayouts every streamed tensor so each DMA reads a single
    fully-contiguous 8-16KB run per partition (the natural [.,.,slice]
    patterns read 256-512B segments, which kept the DMA queues ~25%
    under rate and stalled the weight streams):
      x:  [128, N_TT, N_H, TT]  (one tile per t-tile; first matmul only
           waits on t-tile 0's x)
      wg/wu: [128, N_DG, N_H, WGD]
      wd: [128, N_H, N_D, 128]
  - Loop over T in tiles of TT=512 tokens (one PSUM bank per matmul;
    contiguous accumulation groups -- bank alternation between
    consecutive matmuls measurably breaks LDWEIGHTS pull-ahead):
      stage 1: per d-tile (128 wide): psum_g/psum_u [128, 512] accumulate
        16 matmuls over h-chunks (lhsT = W[h,d] 128x128 stationary,
        rhs = x[h, t-tile] 128x512 moving). silu on ScalarE,
        f = silu(g)*u on VectorE -> f[dt] SBUF [128(d) x 512(t)] bf16.
      stage 2: computed as y^T: per 128-wide h-block, psum_y [128(h) x
        512(t)] accumulates 32 matmuls over d (lhsT = wd[d,h] 128x128
        stationary, rhs = f[dt] 128x512 moving). ScalarE copy -> DMA to
        y [H, T] (y^T layout; host transposes back).
  - DMA queue split: wg/wu on the sync (SP) HW queue; x/wd/y on the
    scalar (ACT) HW queue. (gpsimd's queue is SWDGE -- measured 3x
    slower triggers, do not use.)
"""

import os
import sys

for _p in ("/opt/trn_rl_repo",):
    if _p not in sys.path and os.path.isdir(_p):
        sys.path.insert(0, _p)

import numpy as np
from ml_dtypes import bfloat16 as bf16

E = 8
T = 2048
H = 2048
D = 4096
P = 128
TT = 512
WGD = 256
N_H = H // P        # 16
N_D = D // P        # 32
N_TT = T // TT      # 4
N_DG = D // WGD     # 16

_CACHE = {}


def _build_bass():
    """Build the single-core Bass module (same program for all 8 cores)."""
    import concourse.bass as bass
    import concourse.mybir as mybir
    from concourse.tile import TileContext

    f32 = mybir.dt.float32
    bf = mybir.dt.bfloat16

    nc = bass.Bass(trn_type="TRN2")

    xT = nc.declare_dram_parameter("xT", [P, N_TT * N_H * TT], bf, isOutput=False)
    wg = nc.declare_dram_parameter("wg", [P, N_DG * N_H * WGD], bf, isOutput=False)
    wu = nc.declare_dram_parameter("wu", [P, N_DG * N_H * WGD], bf, isOutput=False)
    wd = nc.declare_dram_parameter("wd", [P, N_H * N_D * P], bf, isOutput=False)
    y = nc.declare_dram_parameter("y", [H, T], f32, isOutput=True)  # y^T

    x_r = xT[:].rearrange("p (t n c) -> p t n c", t=N_TT, n=N_H)
    wg_r = wg[:].rearrange("p (g n c) -> p g n c", g=N_DG, n=N_H)
    wu_r = wu[:].rearrange("p (g n c) -> p g n c", g=N_DG, n=N_H)
    wd_r = wd[:].rearrange("p (m n h) -> p m n h", m=N_H, n=N_D)
    y_r = y[:].rearrange("(n p) t -> p n t", p=P)      # [128, N_H, T]

    with TileContext(nc) as tc:
        with (
            tc.tile_pool(name="xpool", bufs=1) as xpool,
            tc.tile_pool(name="w0pool", bufs=1) as w0pool,
            tc.tile_pool(name="wpool", bufs=3) as wpool,
            tc.tile_pool(name="wdpool", bufs=3) as wdpool,
            tc.tile_pool(name="fpool", bufs=N_D) as fpool,
            tc.tile_pool(name="spool", bufs=2) as spool,
            tc.tile_pool(name="ypool", bufs=4) as ypool,
            tc.tile_pool(name="pgu", bufs=2, space="PSUM") as pgu,
            tc.tile_pool(name="py", bufs=4, space="PSUM") as py,
        ):
            # ---- load x once, one tile per t-tile (t-tile 0 first so
            # stage 1's first matmuls only wait for 2MB, not all 8.4MB).
            # t-tile 0 is further split in h-halves: the first matmul
            # then waits on only 1MB of x + 0.5MB of wg.
            x0a = xpool.tile([P, N_H // 2, TT], bf, tag="x0a")
            x0b = xpool.tile([P, N_H // 2, TT], bf, tag="x0b")
            nc.scalar.dma_start(out=x0a, in_=x_r[:, 0, 0:N_H // 2])
            nc.scalar.dma_start(out=x0b, in_=x_r[:, 0, N_H // 2:N_H])
            # x for t-tiles >= 1 is loaded later (during stage 2 of the
            # previous t-tile) to keep the scalar queue free at startup
            x_tiles = [None] + [None] * (N_TT - 1)

            def x_rhs(tt, h):
                if tt == 0:
                    return (x0a if h < N_H // 2 else x0b)[:, h % (N_H // 2), :]
                return x_tiles[tt][:, h, :]

            # first wg/wu group (t-tile 0, dg 0) in h-halves, own pool
            # (bufs=1), so the first matmuls wait on 0.5MB weight loads
            HH = N_H // 2
            wg0a = w0pool.tile([P, HH, WGD], bf, tag="wg0a")
            wg0b = w0pool.tile([P, HH, WGD], bf, tag="wg0b")
            wu0a = w0pool.tile([P, HH, WGD], bf, tag="wu0a")
            wu0b = w0pool.tile([P, HH, WGD], bf, tag="wu0b")
            nc.sync.dma_start(out=wg0a, in_=wg_r[:, 0, 0:HH])
            nc.sync.dma_start(out=wg0b, in_=wg_r[:, 0, HH:N_H])
            nc.sync.dma_start(out=wu0a, in_=wu_r[:, 0, 0:HH])
            nc.sync.dma_start(out=wu0b, in_=wu_r[:, 0, HH:N_H])

            for tt in range(N_TT):
                tsl = slice(tt * TT, (tt + 1) * TT)

                # ---- stage 1: gate/up + swiglu, d-tile at a time
                f_tiles = []
                for dt in range(N_D):
                    dw = dt % (WGD // P)   # position inside current weight load
                    dg = dt // (WGD // P)
                    first_group = (tt == 0 and dg == 0)
                    if dw == 0 and not first_group:
                        wg_t = wpool.tile([P, N_H, WGD], bf, tag="wg")
                        wu_t = wpool.tile([P, N_H, WGD], bf, tag="wu")
                        nc.sync.dma_start(out=wg_t, in_=wg_r[:, dg])
                        nc.sync.dma_start(out=wu_t, in_=wu_r[:, dg])

                    def wlhs(w_t, wa, wb, h):
                        if first_group:
                            return (wa if h < HH else wb)[
                                :, h % HH, dw * P:(dw + 1) * P]
                        return w_t[:, h, dw * P:(dw + 1) * P]

                    psum_g = pgu.tile([P, TT], f32, tag="pg")
                    psum_u = pgu.tile([P, TT], f32, tag="pu")
                    for h in range(N_H):
                        nc.tensor.matmul(
                            psum_g,
                            lhsT=wlhs(None if first_group else wg_t,
                                      wg0a, wg0b, h),
                            rhs=x_rhs(tt, h),
                            start=(h == 0), stop=(h == N_H - 1),
                        )
                    for h in range(N_H):
                        nc.tensor.matmul(
                            psum_u,
                            lhsT=wlhs(None if first_group else wu_t,
                                      wu0a, wu0b, h),
                            rhs=x_rhs(tt, h),
                            start=(h == 0), stop=(h == N_H - 1),
                        )
                    s_t = spool.tile([P, TT], f32, tag="s")
                    nc.scalar.activation(
                        out=s_t, in_=psum_g,
                        func=mybir.ActivationFunctionType.Silu,
                    )
                    f_t = fpool.tile([P, TT], bf, tag="f")
                    nc.vector.tensor_mul(f_t, s_t, psum_u)
                    f_tiles.append(f_t)

                # ---- stage 2: y^T[hb] = sum_dt wd[dt, hb].T @ f[dt]
                # prefetch next t-tile's x now (needed ~100us later)
                if tt + 1 < N_TT:
                    x_t = xpool.tile([P, N_H, TT], bf, tag=f"x{tt + 1}")
                    nc.scalar.dma_start(out=x_t, in_=x_r[:, tt + 1])
                    x_tiles[tt + 1] = x_t
                for hb in range(N_H):
                    wd_t = wdpool.tile([P, N_D, P], bf, tag="wd")
                    nc.scalar.dma_start(out=wd_t, in_=wd_r[:, hb])
                    psum_y = py.tile([P, TT], f32, tag="py")
                    for dt in range(N_D):
                        nc.tensor.matmul(
                            psum_y,
                            lhsT=wd_t[:, dt, :],
                            rhs=f_tiles[dt][:, :],
                            start=(dt == 0), stop=(dt == N_D - 1),
                        )
                    y_sb = ypool.tile([P, TT], f32, tag="y")
                    nc.scalar.copy(out=y_sb, in_=psum_y)
                    # last t-tile: sync queue is idle (no more wg/wu),
                    # route y stores there so the final drain is fast
                    dma_eng = nc.sync if tt == N_TT - 1 else nc.scalar
                    dma_eng.dma_start(out=y_r[:, hb, tsl], in_=y_sb)
    _split_matmul_waits(nc)
    return nc


def _split_matmul_waits(nc):
    """walrus splits Matmult into LDW+MM and moves the Matmult's sync
    waits onto the generated LW struct, which has room for only one wait.
    Hoist every Matmult's waits onto a PE InstNoOp inserted just before it."""
    import concourse.mybir as mybir

    for f in nc.m.functions:
        for bb in f.blocks:
            insts = list(bb.instructions)
            out = []
            n_nops = 0
            for ins in insts:
                si = ins.sync_info
                tname = type(ins).__name__
                if (
                    si is not None
                    and len(si.on_wait) > (1 if tname != "InstMatmult" else 0)
                ):
                    keep = [] if tname == "InstMatmult" else [si.on_wait[-1]]
                    hoist = si.on_wait if tname == "InstMatmult" else si.on_wait[:-1]
                    for i, w in enumerate(hoist):
                        nop = mybir.InstNoOp(
                            name=f"{ins.name}-waitnop{i}",
                            engine=ins.engine,
                            ins=[],
                            outs=[],
                            sync_info=mybir.SyncInfo(
                                on_wait=[w], on_update=[]
                            ),
                        )
                        out.append(nop)
                        n_nops += 1
                    ins.sync_info = mybir.SyncInfo(
                        on_wait=keep, on_update=list(si.on_update)
                    )
                out.append(ins)
            if n_nops:
                bb.instructions = out


def make_in_maps(hidden_states, gate_proj, up_proj, down_proj):
    hs = np.ascontiguousarray(hidden_states, dtype=np.float32).reshape(E, T, H)
    in_maps = []
    for e in range(E):
        # x: [H, T] -> (n,p,tt,tc) -> (p,tt,n,tc), contiguous per partition
        xh = (hs[e].T.reshape(N_H, P, N_TT, TT)
              .transpose(1, 2, 0, 3).reshape(P, -1))
        # wg/wu: [H, D] -> (n,p,dg,dc) -> (p,dg,n,dc)
        wgh = (np.asarray(gate_proj[e], dtype=np.float32)
               .reshape(N_H, P, N_DG, WGD).transpose(1, 2, 0, 3).reshape(P, -1))
        wuh = (np.asarray(up_proj[e], dtype=np.float32)
               .reshape(N_H, P, N_DG, WGD).transpose(1, 2, 0, 3).reshape(P, -1))
        # wd: [D, H] -> (dt,p,hb,hc) -> (p,hb,dt,hc)
        wdh = (np.asarray(down_proj[e], dtype=np.float32)
               .reshape(N_D, P, N_H, P).transpose(1, 2, 0, 3).reshape(P, -1))
        in_maps.append({
            "xT": np.ascontiguousarray(xh).astype(bf16),
            "wg": np.ascontiguousarray(wgh).astype(bf16),
            "wu": np.ascontiguousarray(wuh).astype(bf16),
            "wd": np.ascontiguousarray(wdh).astype(bf16),
        })
    return in_maps


def kernel(hidden_states, gate_proj, up_proj, down_proj):
    from concourse.bass_utils import run_bass_kernel_spmd

    in_maps = make_in_maps(hidden_states, gate_proj, up_proj, down_proj)
    if "nc" not in _CACHE:
        _CACHE["nc"] = _build_bass()
    nc = _CACHE["nc"]

    res = run_bass_kernel_spmd(nc, in_maps, core_ids=list(range(E)))
    # y comes back as y^T [H, T] per expert
    out = np.concatenate(
        [np.ascontiguousarray(res.results[e]["y"].T) for e in range(E)], axis=0
    )
    return out.astype(np.float32)


if __name__ == "__main__":
    # smoke: build only
    nc = _build_bass()
    print("built ok, instructions:", len(nc.inst_map))


# revision 30
# speedup vs baseline: 1.0440x; 1.0001x over previous
"""Trainium2 Bass kernel for Llama4TextExperts (MoE expert MLP chain).

Problem: E=8 experts, T=2048 tokens/expert, H=2048 hidden, D=4096 intermediate.
  hs (E*T, H) -> per expert e: g = hs_e @ Wg_e; u = hs_e @ Wu_e;
  f = u * silu(g); y_e = f @ Wd_e  -> out (E*T, H), all fp32.

Sharding: expert-parallel, 1 expert per NeuronCore (8 cores).

Per-core kernel design (v10):
  - All matmul operands bf16 (measured rel err ~3.7e-3 vs fp64; gate 2e-2).
  - HOST pre-relayouts every streamed tensor so each DMA reads a single
    fully-contiguous 8-16KB run per partition (the natural [.,.,slice]
    patterns read 256-512B segments, which kept the DMA queues ~25%
    under rate and stalled the weight streams):
      x:  [128, N_TT, N_H, TT]  (one tile per t-tile; first matmul only
           waits on t-tile 0's x)
      wg/wu: [128, N_DG, N_H, WGD]
      wd: [128, N_H, N_D, 128]
  - Loop over T in tiles of TT=512 tokens (one PSUM bank per matmul;
    contiguous accumulation groups -- bank alternation between
    consecutive matmuls measurably breaks LDWEIGHTS pull-ahead):
      stage 1: per d-tile (128 wide): psum_g/psum_u [128, 512] accumulate
        16 matmuls over h-chunks (lhsT = W[h,d] 128x128 stationary,
        rhs = x[h, t-tile] 128x512 moving). silu on ScalarE,
        f = silu(g)*u on VectorE -> f[dt] SBUF [128(d) x 512(t)] bf16.
      stage 2: computed as y^T: per 128-wide h-block, psum_y [128(h) x
        512(t)] accumulates 32 matmuls over d (lhsT = wd[d,h] 128x128
        stationary, rhs = f[dt] 128x512 moving). ScalarE copy -> DMA to
        y [H, T] (y^T layout; host transposes back).
  - DMA queue split: wg/wu on the sync (SP) HW queue; x/wd/y on the
    scalar (ACT) HW queue. (gpsimd's queue is SWDGE -- measured 3x
    slower triggers, do not use.)
"""

import os
import sys

for _p in ("/opt/trn_rl_repo",):
    if _p not in sys.path and os.path.isdir(_p):
        sys.path.insert(0, _p)

import numpy as np
from ml_dtypes import bfloat16 as bf16

E = 8
T = 2048
H = 2048
D = 4096
P = 128
TT = 512
WGD = 256
N_H = H // P        # 16
N_D = D // P        # 32
N_TT = T // TT      # 4
N_DG = D // WGD     # 16

_CACHE = {}


def _build_bass():
    """Build the single-core Bass module (same program for all 8 cores)."""
    import concourse.bass as bass
    import concourse.mybir as mybir
    from concourse.tile import TileContext

    f32 = mybir.dt.float32
    bf = mybir.dt.bfloat16

    nc = bass.Bass(trn_type="TRN2")

    xT = nc.declare_dram_parameter("xT", [P, N_TT * N_H * TT], bf, isOutput=False)
    wg = nc.declare_dram_parameter("wg", [P, N_DG * N_H * WGD], bf, isOutput=False)
    wu = nc.declare_dram_parameter("wu", [P, N_DG * N_H * WGD], bf, isOutput=False)
    wd = nc.declare_dram_parameter("wd", [P, N_H * N_D * P], bf, isOutput=False)
    y = nc.declare_dram_parameter("y", [H, T], f32, isOutput=True)  # y^T

    x_r = xT[:].rearrange("p (t n c) -> p t n c", t=N_TT, n=N_H)
    wg_r = wg[:].rearrange("p (g n c) -> p g n c", g=N_DG, n=N_H)
    wu_r = wu[:].rearrange("p (g n c) -> p g n c", g=N_DG, n=N_H)
    wd_r = wd[:].rearrange("p (m n h) -> p m n h", m=N_H, n=N_D)
    y_r = y[:].rearrange("(n p) t -> p n t", p=P)      # [128, N_H, T]

    with TileContext(nc) as tc:
        with (
            tc.tile_pool(name="xpool", bufs=1) as xpool,
            tc.tile_pool(name="w0pool", bufs=1) as w0pool,
            tc.tile_pool(name="wpool", bufs=3) as wpool,
            tc.tile_pool(name="wdpool", bufs=3) as wdpool,
            tc.tile_pool(name="fpool", bufs=N_D) as fpool,
            tc.tile_pool(name="spool", bufs=2) as spool,
            tc.tile_pool(name="ypool", bufs=4) as ypool,
            tc.tile_pool(name="pgu", bufs=2, space="PSUM") as pgu,
            tc.tile_pool(name="py", bufs=4, space="PSUM") as py,
        ):
            # ---- load x once, one tile per t-tile (t-tile 0 first so
            # stage 1's first matmuls only wait for 2MB, not all 8.4MB).
            # t-tile 0 is further split in h-halves: the first matmul
            # then waits on only 1MB of x + 0.5MB of wg.
            NQ = 4           # t-tile 0 x/w tiles split in quarters
            HQ = N_H // NQ
            x0q = []
            for q in range(NQ):
                x0 = xpool.tile([P, HQ, TT], bf, tag=f"x0q{q}")
                nc.scalar.dma_start(out=x0, in_=x_r[:, 0, q * HQ:(q + 1) * HQ])
                x0q.append(x0)
            # x for t-tiles >= 1 is loaded later (during stage 2 of the
            # previous t-tile) to keep the scalar queue free at startup
            x_tiles = [None] + [None] * (N_TT - 1)

            def x_rhs(tt, h):
                if tt == 0:
                    return x0q[h // HQ][:, h % HQ, :]
                return x_tiles[tt][:, h, :]

            # first wg/wu group (t-tile 0, dg 0) in h-quarters, own pool
            # (bufs=1), so the first matmuls wait on 0.25MB weight loads
            wg0q, wu0q = [], []
            for q in range(NQ):
                wg0 = w0pool.tile([P, HQ, WGD], bf, tag=f"wg0q{q}")
                nc.sync.dma_start(out=wg0, in_=wg_r[:, 0, q * HQ:(q + 1) * HQ])
                wg0q.append(wg0)
            for q in range(NQ):
                wu0 = w0pool.tile([P, HQ, WGD], bf, tag=f"wu0q{q}")
                nc.sync.dma_start(out=wu0, in_=wu_r[:, 0, q * HQ:(q + 1) * HQ])
                wu0q.append(wu0)

            for tt in range(N_TT):
                tsl = slice(tt * TT, (tt + 1) * TT)

                # ---- stage 1: gate/up + swiglu, d-tile at a time
                f_tiles = []
                for dt in range(N_D):
                    dw = dt % (WGD // P)   # position inside current weight load
                    dg = dt // (WGD // P)
                    first_group = (tt == 0 and dg == 0)
                    if dw == 0 and not first_group:
                        wg_t = wpool.tile([P, N_H, WGD], bf, tag="wg")
                        wu_t = wpool.tile([P, N_H, WGD], bf, tag="wu")
                        nc.sync.dma_start(out=wg_t, in_=wg_r[:, dg])
                        nc.sync.dma_start(out=wu_t, in_=wu_r[:, dg])

                    def wlhs(w_t, wq, h):
                        if first_group:
                            return wq[h // HQ][:, h % HQ, dw * P:(dw + 1) * P]
                        return w_t[:, h, dw * P:(dw + 1) * P]

                    psum_g = pgu.tile([P, TT], f32, tag="pg")
                    psum_u = pgu.tile([P, TT], f32, tag="pu")
                    for h in range(N_H):
                        nc.tensor.matmul(
                            psum_g,
                            lhsT=wlhs(None if first_group else wg_t,
                                      wg0q, h),
                            rhs=x_rhs(tt, h),
                            start=(h == 0), stop=(h == N_H - 1),
                        )
                    for h in range(N_H):
                        nc.tensor.matmul(
                            psum_u,
                            lhsT=wlhs(None if first_group else wu_t,
                                      wu0q, h),
                            rhs=x_rhs(tt, h),
                            start=(h == 0), stop=(h == N_H - 1),
                        )
                    s_t = spool.tile([P, TT], f32, tag="s")
                    nc.scalar.activation(
                        out=s_t, in_=psum_g,
                        func=mybir.ActivationFunctionType.Silu,
                    )
                    f_t = fpool.tile([P, TT], bf, tag="f")
                    nc.vector.tensor_mul(f_t, s_t, psum_u)
                    f_tiles.append(f_t)

                # ---- stage 2: y^T[hb] = sum_dt wd[dt, hb].T @ f[dt]
                # prefetch next t-tile's x now (needed ~100us later)
                if tt + 1 < N_TT:
                    x_t = xpool.tile([P, N_H, TT], bf, tag=f"x{tt + 1}")
                    nc.scalar.dma_start(out=x_t, in_=x_r[:, tt + 1])
                    x_tiles[tt + 1] = x_t
                for hb in range(N_H):
                    wd_t = wdpool.tile([P, N_D, P], bf, tag="wd")
                    nc.scalar.dma_start(out=wd_t, in_=wd_r[:, hb])
                    # last t-tile: sync queue is idle (no more wg/wu),
                    # route y stores there so the final drain is fast
                    dma_eng = nc.sync if tt == N_TT - 1 else nc.scalar
                    last_hb = (tt == N_TT - 1 and hb == N_H - 1)
                    if not last_hb:
                        psum_y = py.tile([P, TT], f32, tag="py")
                        for dt in range(N_D):
                            nc.tensor.matmul(
                                psum_y,
                                lhsT=wd_t[:, dt, :],
                                rhs=f_tiles[dt][:, :],
                                start=(dt == 0), stop=(dt == N_D - 1),
                            )
                        y_sb = ypool.tile([P, TT], f32, tag="y")
                        nc.scalar.copy(out=y_sb, in_=psum_y)
                        dma_eng.dma_start(out=y_r[:, hb, tsl], in_=y_sb)
                    else:
                        # very last output block: accumulate the two
                        # token-halves into separate PSUM banks serially,
                        # so half A's copy+DMA overlaps half B's matmuls
                        # and the post-last-matmul drain is ~1us not ~4us
                        for half in range(2):
                            csl = slice(half * (TT // 2),
                                        (half + 1) * (TT // 2))
                            psum_y = py.tile([P, TT], f32, tag="py")
                            for dt in range(N_D):
                                nc.tensor.matmul(
                                    psum_y[:, 0:TT // 2],
                                    lhsT=wd_t[:, dt, :],
                                    rhs=f_tiles[dt][:, csl],
                                    start=(dt == 0), stop=(dt == N_D - 1),
                                )
                            y_sb = ypool.tile([P, TT // 2], f32, tag="yl")
                            nc.scalar.copy(out=y_sb, in_=psum_y[:, 0:TT // 2])
                            dma_eng.dma_start(
                                out=y_r[:, hb,
                                        tt * TT + half * (TT // 2):
                                        tt * TT + (half + 1) * (TT // 2)],
                                in_=y_sb,
                            )
    _split_matmul_waits(nc)
    return nc


def _split_matmul_waits(nc):
    """walrus splits Matmult into LDW+MM and moves the Matmult's sync
    waits onto the generated LW struct, which has room for only one wait.
    Hoist every Matmult's waits onto a PE InstNoOp inserted just before it."""
    import concourse.mybir as mybir

    for f in nc.m.functions:
        for bb in f.blocks:
            insts = list(bb.instructions)
            out = []
            n_nops = 0
            for ins in insts:
                si = ins.sync_info
                tname = type(ins).__name__
                if (
                    si is not None
                    and len(si.on_wait) > (1 if tname != "InstMatmult" else 0)
                ):
                    keep = [] if tname == "InstMatmult" else [si.on_wait[-1]]
                    hoist = si.on_wait if tname == "InstMatmult" else si.on_wait[:-1]
                    for i, w in enumerate(hoist):
                        nop = mybir.InstNoOp(
                            name=f"{ins.name}-waitnop{i}",
                            engine=ins.engine,
                            ins=[],
                            outs=[],
                            sync_info=mybir.SyncInfo(
                                on_wait=[w], on_update=[]
                            ),
                        )
                        out.append(nop)
                        n_nops += 1
                    ins.sync_info = mybir.SyncInfo(
                        on_wait=keep, on_update=list(si.on_update)
                    )
                out.append(ins)
            if n_nops:
                bb.instructions = out


def make_in_maps(hidden_states, gate_proj, up_proj, down_proj):
    hs = np.ascontiguousarray(hidden_states, dtype=np.float32).reshape(E, T, H)
    in_maps = []
    for e in range(E):
        # x: [H, T] -> (n,p,tt,tc) -> (p,tt,n,tc), contiguous per partition
        xh = (hs[e].T.reshape(N_H, P, N_TT, TT)
              .transpose(1, 2, 0, 3).reshape(P, -1))
        # wg/wu: [H, D] -> (n,p,dg,dc) -> (p,dg,n,dc)
        wgh = (np.asarray(gate_proj[e], dtype=np.float32)
               .reshape(N_H, P, N_DG, WGD).transpose(1, 2, 0, 3).reshape(P, -1))
        wuh = (np.asarray(up_proj[e], dtype=np.float32)
               .reshape(N_H, P, N_DG, WGD).transpose(1, 2, 0, 3).reshape(P, -1))
        # wd: [D, H] -> (dt,p,hb,hc) -> (p,hb,dt,hc)
        wdh = (np.asarray(down_proj[e], dtype=np.float32)
               .reshape(N_D, P, N_H, P).transpose(1, 2, 0, 3).reshape(P, -1))
        in_maps.append({
            "xT": np.ascontiguousarray(xh).astype(bf16),
            "wg": np.ascontiguousarray(wgh).astype(bf16),
            "wu": np.ascontiguousarray(wuh).astype(bf16),
            "wd": np.ascontiguousarray(wdh).astype(bf16),
        })
    return in_maps


def kernel(hidden_states, gate_proj, up_proj, down_proj):
    from concourse.bass_utils import run_bass_kernel_spmd

    in_maps = make_in_maps(hidden_states, gate_proj, up_proj, down_proj)
    if "nc" not in _CACHE:
        _CACHE["nc"] = _build_bass()
    nc = _CACHE["nc"]

    res = run_bass_kernel_spmd(nc, in_maps, core_ids=list(range(E)))
    # y comes back as y^T [H, T] per expert
    out = np.concatenate(
        [np.ascontiguousarray(res.results[e]["y"].T) for e in range(E)], axis=0
    )
    return out.astype(np.float32)


if __name__ == "__main__":
    # smoke: build only
    nc = _build_bass()
    print("built ok, instructions:", len(nc.inst_map))


# revision 31
# speedup vs baseline: 1.0473x; 1.0032x over previous
"""Trainium2 Bass kernel for Llama4TextExperts (MoE expert MLP chain).

Problem: E=8 experts, T=2048 tokens/expert, H=2048 hidden, D=4096 intermediate.
  hs (E*T, H) -> per expert e: g = hs_e @ Wg_e; u = hs_e @ Wu_e;
  f = u * silu(g); y_e = f @ Wd_e  -> out (E*T, H), all fp32.

Sharding: expert-parallel, 1 expert per NeuronCore (8 cores).

Per-core kernel design (final; measured 1.353ms HW, vs 1.392ms staged
baseline and a 1.311ms pure-matmul-streaming floor):
  - All matmul operands bf16 (measured rel err ~3.7e-3 vs fp64; gate 2e-2).
  - HOST pre-relayouts every streamed tensor so each DMA reads a single
    fully-contiguous 8-16KB run per partition (the natural [.,.,slice]
    patterns read 256-512B segments, which kept the DMA queues ~25%
    under rate and stalled the weight streams):
      x:  [128, N_TT, N_H, TT]  (one tile per t-tile; first matmul only
           waits on t-tile 0's x)
      wg/wu: [128, N_DG, N_H, WGD]
      wd: [128, N_H, N_D, 128]
  - Loop over T in tiles of TT=512 tokens (one PSUM bank per matmul;
    contiguous accumulation groups -- bank alternation between
    consecutive matmuls measurably breaks LDWEIGHTS pull-ahead):
      stage 1: per d-tile (128 wide): psum_g/psum_u [128, 512] accumulate
        16 matmuls over h-chunks (lhsT = W[h,d] 128x128 stationary,
        rhs = x[h, t-tile] 128x512 moving). silu on ScalarE,
        f = silu(g)*u on VectorE -> f[dt] SBUF [128(d) x 512(t)] bf16.
      stage 2: computed as y^T: per 128-wide h-block, psum_y [128(h) x
        512(t)] accumulates 32 matmuls over d (lhsT = wd[d,h] 128x128
        stationary, rhs = f[dt] 128x512 moving). ScalarE copy -> DMA to
        y [H, T] (y^T layout; host transposes back).
  - DMA queue split: wg/wu on the sync (SP) HW queue; x/wd/y on the
    scalar (ACT) HW queue. (gpsimd's queue is SWDGE -- measured 3x
    slower triggers, do not use.)
"""

import os
import sys

for _p in ("/opt/trn_rl_repo",):
    if _p not in sys.path and os.path.isdir(_p):
        sys.path.insert(0, _p)

import numpy as np
from ml_dtypes import bfloat16 as bf16

E = 8
T = 2048
H = 2048
D = 4096
P = 128
TT = 512
WGD = 256
N_H = H // P        # 16
N_D = D // P        # 32
N_TT = T // TT      # 4
N_DG = D // WGD     # 16

_CACHE = {}


def _build_bass():
    """Build the single-core Bass module (same program for all 8 cores)."""
    import concourse.bass as bass
    import concourse.mybir as mybir
    from concourse.tile import TileContext

    f32 = mybir.dt.float32
    bf = mybir.dt.bfloat16

    nc = bass.Bass(trn_type="TRN2")

    xT = nc.declare_dram_parameter("xT", [P, N_TT * N_H * TT], bf, isOutput=False)
    wg = nc.declare_dram_parameter("wg", [P, N_DG * N_H * WGD], bf, isOutput=False)
    wu = nc.declare_dram_parameter("wu", [P, N_DG * N_H * WGD], bf, isOutput=False)
    wd = nc.declare_dram_parameter("wd", [P, N_H * N_D * P], bf, isOutput=False)
    y = nc.declare_dram_parameter("y", [H, T], f32, isOutput=True)  # y^T

    x_r = xT[:].rearrange("p (t n c) -> p t n c", t=N_TT, n=N_H)
    wg_r = wg[:].rearrange("p (g n c) -> p g n c", g=N_DG, n=N_H)
    wu_r = wu[:].rearrange("p (g n c) -> p g n c", g=N_DG, n=N_H)
    wd_r = wd[:].rearrange("p (m n h) -> p m n h", m=N_H, n=N_D)
    y_r = y[:].rearrange("(n p) t -> p n t", p=P)      # [128, N_H, T]

    with TileContext(nc) as tc:
        with (
            tc.tile_pool(name="xpool", bufs=1) as xpool,
            tc.tile_pool(name="w0pool", bufs=1) as w0pool,
            tc.tile_pool(name="wpool", bufs=3) as wpool,
            tc.tile_pool(name="wdpool", bufs=3) as wdpool,
            tc.tile_pool(name="fpool", bufs=N_D) as fpool,
            tc.tile_pool(name="spool", bufs=2) as spool,
            tc.tile_pool(name="ypool", bufs=4) as ypool,
            tc.tile_pool(name="pgu", bufs=2, space="PSUM") as pgu,
            tc.tile_pool(name="py", bufs=4, space="PSUM") as py,
        ):
            # ---- load x once, one tile per t-tile (t-tile 0 first so
            # stage 1's first matmuls only wait for 2MB, not all 8.4MB).
            # t-tile 0 is further split in h-halves: the first matmul
            # then waits on only 1MB of x + 0.5MB of wg.
            NQ = 4           # t-tile 0 x/w tiles split in quarters
            HQ = N_H // NQ
            x0q = []
            for q in range(NQ):
                x0 = xpool.tile([P, HQ, TT], bf, tag=f"x0q{q}")
                nc.scalar.dma_start(out=x0, in_=x_r[:, 0, q * HQ:(q + 1) * HQ])
                x0q.append(x0)
            # x for t-tiles >= 1 is loaded later (during stage 2 of the
            # previous t-tile) to keep the scalar queue free at startup
            x_tiles = [None] + [None] * (N_TT - 1)

            def x_rhs(tt, h):
                if tt == 0:
                    return x0q[h // HQ][:, h % HQ, :]
                return x_tiles[tt][:, h, :]

            # first wg/wu group (t-tile 0, dg 0) in h-quarters, own pool
            # (bufs=1), so the first matmuls wait on 0.25MB weight loads
            wg0q, wu0q = [], []
            for q in range(NQ):
                wg0 = w0pool.tile([P, HQ, WGD], bf, tag=f"wg0q{q}")
                nc.sync.dma_start(out=wg0, in_=wg_r[:, 0, q * HQ:(q + 1) * HQ])
                wg0q.append(wg0)
            for q in range(NQ):
                wu0 = w0pool.tile([P, HQ, WGD], bf, tag=f"wu0q{q}")
                nc.sync.dma_start(out=wu0, in_=wu_r[:, 0, q * HQ:(q + 1) * HQ])
                wu0q.append(wu0)

            for tt in range(N_TT):
                tsl = slice(tt * TT, (tt + 1) * TT)

                # ---- stage 1: gate/up + swiglu, d-tile at a time
                f_tiles = []
                for dt in range(N_D):
                    dw = dt % (WGD // P)   # position inside current weight load
                    dg = dt // (WGD // P)
                    first_group = (tt == 0 and dg == 0)
                    if dw == 0 and not first_group:
                        wg_t = wpool.tile([P, N_H, WGD], bf, tag="wg")
                        wu_t = wpool.tile([P, N_H, WGD], bf, tag="wu")
                        nc.sync.dma_start(out=wg_t, in_=wg_r[:, dg])
                        nc.sync.dma_start(out=wu_t, in_=wu_r[:, dg])

                    def wlhs(w_t, wq, h):
                        if first_group:
                            return wq[h // HQ][:, h % HQ, dw * P:(dw + 1) * P]
                        return w_t[:, h, dw * P:(dw + 1) * P]

                    psum_g = pgu.tile([P, TT], f32, tag="pg")
                    psum_u = pgu.tile([P, TT], f32, tag="pu")
                    for h in range(N_H):
                        nc.tensor.matmul(
                            psum_g,
                            lhsT=wlhs(None if first_group else wg_t,
                                      wg0q, h),
                            rhs=x_rhs(tt, h),
                            start=(h == 0), stop=(h == N_H - 1),
                        )
                    for h in range(N_H):
                        nc.tensor.matmul(
                            psum_u,
                            lhsT=wlhs(None if first_group else wu_t,
                                      wu0q, h),
                            rhs=x_rhs(tt, h),
                            start=(h == 0), stop=(h == N_H - 1),
                        )
                    s_t = spool.tile([P, TT], f32, tag="s")
                    nc.scalar.activation(
                        out=s_t, in_=psum_g,
                        func=mybir.ActivationFunctionType.Silu,
                    )
                    f_t = fpool.tile([P, TT], bf, tag="f")
                    nc.vector.tensor_mul(f_t, s_t, psum_u)
                    f_tiles.append(f_t)

                # ---- stage 2: y^T[hb] = sum_dt wd[dt, hb].T @ f[dt]
                # prefetch next t-tile's x now (needed ~100us later)
                if tt + 1 < N_TT:
                    x_t = xpool.tile([P, N_H, TT], bf, tag=f"x{tt + 1}")
                    nc.scalar.dma_start(out=x_t, in_=x_r[:, tt + 1])
                    x_tiles[tt + 1] = x_t
                for hb in range(N_H):
                    wd_t = wdpool.tile([P, N_D, P], bf, tag="wd")
                    nc.scalar.dma_start(out=wd_t, in_=wd_r[:, hb])
                    # last t-tile: sync queue is idle (no more wg/wu),
                    # route y stores there so the final drain is fast
                    dma_eng = nc.sync if tt == N_TT - 1 else nc.scalar
                    last_hb = (tt == N_TT - 1 and hb == N_H - 1)
                    if not last_hb:
                        psum_y = py.tile([P, TT], f32, tag="py")
                        for dt in range(N_D):
                            nc.tensor.matmul(
                                psum_y,
                                lhsT=wd_t[:, dt, :],
                                rhs=f_tiles[dt][:, :],
                                start=(dt == 0), stop=(dt == N_D - 1),
                            )
                        y_sb = ypool.tile([P, TT], f32, tag="y")
                        nc.scalar.copy(out=y_sb, in_=psum_y)
                        dma_eng.dma_start(out=y_r[:, hb, tsl], in_=y_sb)
                    else:
                        # very last output block: accumulate the two
                        # token-halves into separate PSUM banks serially,
                        # so half A's copy+DMA overlaps half B's matmuls
                        # and the post-last-matmul drain is ~1us not ~4us
                        for half in range(2):
                            csl = slice(half * (TT // 2),
                                        (half + 1) * (TT // 2))
                            psum_y = py.tile([P, TT], f32, tag="py")
                            for dt in range(N_D):
                                nc.tensor.matmul(
                                    psum_y[:, 0:TT // 2],
                                    lhsT=wd_t[:, dt, :],
                                    rhs=f_tiles[dt][:, csl],
                                    start=(dt == 0), stop=(dt == N_D - 1),
                                )
                            y_sb = ypool.tile([P, TT // 2], f32, tag="yl")
                            nc.scalar.copy(out=y_sb, in_=psum_y[:, 0:TT // 2])
                            dma_eng.dma_start(
                                out=y_r[:, hb,
                                        tt * TT + half * (TT // 2):
                                        tt * TT + (half + 1) * (TT // 2)],
                                in_=y_sb,
                            )
    _split_matmul_waits(nc)
    return nc


def _split_matmul_waits(nc):
    """walrus splits Matmult into LDW+MM and moves the Matmult's sync
    waits onto the generated LW struct, which has room for only one wait.
    Hoist every Matmult's waits onto a PE InstNoOp inserted just before it."""
    import concourse.mybir as mybir

    for f in nc.m.functions:
        for bb in f.blocks:
            insts = list(bb.instructions)
            out = []
            n_nops = 0
            for ins in insts:
                si = ins.sync_info
                tname = type(ins).__name__
                if (
                    si is not None
                    and len(si.on_wait) > (1 if tname != "InstMatmult" else 0)
                ):
                    keep = [] if tname == "InstMatmult" else [si.on_wait[-1]]
                    hoist = si.on_wait if tname == "InstMatmult" else si.on_wait[:-1]
                    for i, w in enumerate(hoist):
                        nop = mybir.InstNoOp(
                            name=f"{ins.name}-waitnop{i}",
                            engine=ins.engine,
                            ins=[],
                            outs=[],
                            sync_info=mybir.SyncInfo(
                                on_wait=[w], on_update=[]
                            ),
                        )
                        out.append(nop)
                        n_nops += 1
                    ins.sync_info = mybir.SyncInfo(
                        on_wait=keep, on_update=list(si.on_update)
                    )
                out.append(ins)
            if n_nops:
                bb.instructions = out


def make_in_maps(hidden_states, gate_proj, up_proj, down_proj):
    hs = np.ascontiguousarray(hidden_states, dtype=np.float32).reshape(E, T, H)
    in_maps = []
    for e in range(E):
        # x: [H, T] -> (n,p,tt,tc) -> (p,tt,n,tc), contiguous per partition
        xh = (hs[e].T.reshape(N_H, P, N_TT, TT)
              .transpose(1, 2, 0, 3).reshape(P, -1))
        # wg/wu: [H, D] -> (n,p,dg,dc) -> (p,dg,n,dc)
        wgh = (np.asarray(gate_proj[e], dtype=np.float32)
               .reshape(N_H, P, N_DG, WGD).transpose(1, 2, 0, 3).reshape(P, -1))
        wuh = (np.asarray(up_proj[e], dtype=np.float32)
               .reshape(N_H, P, N_DG, WGD).transpose(1, 2, 0, 3).reshape(P, -1))
        # wd: [D, H] -> (dt,p,hb,hc) -> (p,hb,dt,hc)
        wdh = (np.asarray(down_proj[e], dtype=np.float32)
               .reshape(N_D, P, N_H, P).transpose(1, 2, 0, 3).reshape(P, -1))
        in_maps.append({
            "xT": np.ascontiguousarray(xh).astype(bf16),
            "wg": np.ascontiguousarray(wgh).astype(bf16),
            "wu": np.ascontiguousarray(wuh).astype(bf16),
            "wd": np.ascontiguousarray(wdh).astype(bf16),
        })
    return in_maps


def kernel(hidden_states, gate_proj, up_proj, down_proj):
    from concourse.bass_utils import run_bass_kernel_spmd

    in_maps = make_in_maps(hidden_states, gate_proj, up_proj, down_proj)
    if "nc" not in _CACHE:
        _CACHE["nc"] = _build_bass()
    nc = _CACHE["nc"]

    res = run_bass_kernel_spmd(nc, in_maps, core_ids=list(range(E)))
    # y comes back as y^T [H, T] per expert
    out = np.concatenate(
        [np.ascontiguousarray(res.results[e]["y"].T) for e in range(E)], axis=0
    )
    return out.astype(np.float32)


if __name__ == "__main__":
    # smoke: build only
    nc = _build_bass()
    print("built ok, instructions:", len(nc.inst_map))
